# revision 36
# baseline (speedup 1.0000x reference)
"""Causal single-head attention (B=4, S=2048, E=1024, D=64) on 8 TRN2 NeuronCores.

Sharding: core c -> batch b = c//2, parity h = c%2; core owns q-blocks of its
parity (h=1 even, h=0 odd; 68/136 causal units each). No collectives.

v2 design (vs v1 baseline at 86.3us):
- Per-core block-PERMUTED xT layout: within each group of 4 blocks the core's
  two owned q-blocks come first. The Q projection then only computes owned
  columns at FIXED offsets (uniform SPMD graph, no select ops), 1/2 the Q work.
  Causal group structure is preserved (permutation stays within groups of 4),
  masks/output mapping carry the permutation in DATA.
- [Wk|Wv] packed as one 128-wide stationary: K^T and V^T computed by a single
  pass over x (psum rows 0:64=K^T, 64:128=V^T), halving the K/V matmul count.
- Group-streamed pipeline: for each 512-token group g: DMA(g+1) || proj(g) ->
  attn(pair g). DMA hides behind compute; PE stays dense (HAM warm).
- V natural layout via dma_start_transpose (DMA engines), not PE transposes.
- exp on [128,512] tiles (2 key blocks per ACTIVATE) to amortize ScalarE
  instruction overhead -- attention was ScalarE-bound in v1.
- Epilogue: ships out^T = [65, 256] per pair (row 64 = softmax denominator);
  final transpose + normalize folded into the host-side unshard
  (flash-decoding style partial-result combine).
"""

import os
import sys

sys.path.insert(0, "/opt/trn_rl_repo")

import numpy as np

B, S, E, D = 4, 2048, 1024, 64
NB = S // 128       # 16 token blocks
NE = E // 128       # 8 contraction chunks
SG = 4              # groups of 4 blocks (512 tokens)
GW = 512            # group width (cols)
OW = 256            # owned q cols per group
NCORES = 8

# within-group block order: owned parity blocks first
PERM_REL = {1: [0, 2, 1, 3], 0: [1, 3, 0, 2]}

_BUILT = {}
LAST = None  # BassKernelResults of the most recent run (for test harness)


def _build():
    variant = os.environ.get("KVARIANT", "full")
    from concourse import bacc, bass, tile, mybir

    f32 = mybir.dt.float32
    bf16 = mybir.dt.bfloat16
    ADD = mybir.AluOpType.add
    EXP = mybir.ActivationFunctionType.Exp

    nc = bacc.Bacc(None, target_bir_lowering=False, debug=False)

    # cb: [wkv | wq | mask] packed; cf: [bkv | bq] packed
    CB_W = NE * 128 + NE * D + 2 * 2 * OW + D  # 1024 + 512 + 1024 + 64 (iden)
    xT_d = nc.declare_dram_parameter("xT", [128, SG * NE * GW], bf16, isOutput=False)
    cb_d = nc.declare_dram_parameter("cb", [128, CB_W], bf16, isOutput=False)
    cf_d = nc.declare_dram_parameter("cf", [128, 2], f32, isOutput=False)
    out_d = nc.declare_dram_parameter("out", [SG, D + 1, OW], f32, isOutput=True)
    if variant == "dump":
        dbg_d = nc.declare_dram_parameter("dbg", [SG, 128, GW + 4 * (D + 1)], bf16, isOutput=True)

    with tile.TileContext(nc) as tc:
        with (
            tc.tile_pool(name="consts", bufs=1) as consts,
            tc.tile_pool(name="xpool", bufs=1) as xpool,
            tc.tile_pool(name="acts", bufs=1) as acts,
            tc.tile_pool(name="probs", bufs=4) as probs_pool,
            tc.tile_pool(name="smalls", bufs=2) as smalls,
            tc.tile_pool(name="ps_a", bufs=3, space="PSUM") as ps_a,
            tc.tile_pool(name="ps_sc", bufs=3, space="PSUM") as ps_sc,
            tc.tile_pool(name="ps_o", bufs=2, space="PSUM") as ps_o,
        ):
            # ---- constants to SBUF: wkv first (gates the first matmul),
            # then group-0 x, then the rest of the constants
            cb = consts.tile([128, CB_W], bf16, tag="cb")
            cf = consts.tile([128, 2], f32, tag="cf")
            wkv = cb[:, 0 : NE * 128]
            wq = cb[:, NE * 128 : NE * 128 + NE * D]
            mask = cb[:, NE * 128 + NE * D : CB_W - D]
            iden = cb[:, CB_W - D : CB_W]
            bkv = cf[:, 0:1]
            bq = cf[0:D, 1:2]
            nc.sync.dma_start(wkv, cb_d[:, 0 : NE * 128])

            # warm the ScalarE exp table during initial DMA wait
            scr0 = smalls.tile([1, 1], f32, tag="scr0")
            scr1 = smalls.tile([1, 1], f32, tag="scr1")
            nc.vector.memset(scr0[:], 0.0)
            nc.scalar.activation(scr1[:], scr0[:], EXP)
            # warm the PE HAM clock gate with dummy matmuls while x loads
            wrm = smalls.tile([128, GW], bf16, tag="wrm")
            nc.vector.memset(wrm[:], 0.0)
            pwm = ps_a.tile([128, GW], f32, tag="ps", name="pwm")
            for i in range(16):
                nc.tensor.matmul(
                    pwm[:], wrm[:, 0:128], wrm[:], start=True, stop=True
                )

            # ---- persistent activations (per-group tiles -> exact deps)
            kvg = []   # [128, GW] bf16: rows 0:64 K^T, 64:128 V^T (permuted order)
            qg = []    # [64, OW] bf16: owned Q^T
            vsbg = []  # [128, 4, D+1] bf16: V natural per block + ones col
            for g in range(SG):
                kvg.append(acts.tile([128, GW], bf16, tag=f"kv{g}", name=f"kv{g}"))
                qg.append(acts.tile([D, OW], bf16, tag=f"q{g}", name=f"q{g}"))
                vsbg.append(acts.tile([128, 4, 128], bf16, tag=f"vsb{g}", name=f"vsb{g}"))

            for g in range(SG):
                nc.vector.memset(vsbg[g][:, :, D : D + 1], 1.0)

            def xg_load(g, nsplit):
                """Emit the load of group g's x^T slice (nsplit DMAs)."""
                xg = xpool.tile([128, NE, GW], bf16, tag=f"xg{g}", name=f"xg{g}")
                step = NE // nsplit
                for s in range(nsplit):
                    nc.sync.dma_start(
                        xg[:, s * step : (s + 1) * step, :],
                        xT_d[
                            :,
                            (g * NE + s * step) * GW : (g * NE + (s + 1) * step) * GW,
                        ],
                    )
                return xg

            xgs = {}
            pouts = {}

            def proj(g):
                """K|V packed pass + owned-Q pass for group g, plus the V
                natural-layout XBAR transpose chain (GpSimd bounce -> SP)."""
                xg = xgs[g]
                pkv = ps_a.tile([128, GW], f32, tag="ps", name=f"pkv{g}")
                for e in range(NE):
                    nc.tensor.matmul(
                        pkv[:],
                        wkv[:, e * 128 : (e + 1) * 128],
                        xg[:, e, :],
                        start=(e == 0),
                        stop=(e == NE - 1),
                    )
                nc.vector.tensor_scalar(kvg[g][:], pkv[:], bkv[:], None, ADD)
                pq = ps_a.tile([128, GW], f32, tag="ps", name=f"pq{g}")
                for e in range(NE):
                    nc.tensor.matmul(
                        pq[0:D, 0:OW],
                        wq[:, e * D : (e + 1) * D],
                        xg[:, e, 0:OW],
                        start=(e == 0),
                        stop=(e == NE - 1),
                    )
                nc.vector.tensor_scalar(qg[g][:], pq[0:D, 0:OW], bq[:], None, ADD)
                # V natural layout via PE transposes (row-group 64-127)
                for j in range(4):
                    ptr = ps_a.tile([128, GW], bf16, tag="ps", name=f"ptr{g}_{j}")
                    nc.tensor.transpose(
                        ptr[0:128, 0:D],
                        kvg[g][D:128, j * 128 : (j + 1) * 128],
                        iden[D:128, :],
                    )
                    nc.vector.tensor_copy(vsbg[g][:, j, 0:D], ptr[0:128, 0:D])
                if variant == "dump":
                    nc.sync.dma_start(dbg_d[g][:, 0:GW], kvg[g][:])
                    for j in range(4):
                        nc.sync.dma_start(
                            dbg_d[g][:, GW + j * (D + 1) : GW + (j + 1) * (D + 1)],
                            vsbg[g][:, j, 0 : D + 1],
                        )

            def cell_scores(p, g):
                """Scores+exp(+mask) of pair p vs key-group g -> pt tiles."""
                pts = []
                for kbp in (2 * g, 2 * g + 1):
                    k0 = (kbp % 2) * 2
                    psc = ps_sc.tile(
                        [128, 2 * OW], f32, tag="psc", name=f"psc{p}_{kbp}"
                    )
                    nc.tensor.matmul(
                        psc[:, 0:OW],
                        kvg[g][0:D, k0 * 128 : (k0 + 1) * 128],
                        qg[p][:],
                        start=True,
                        stop=True,
                    )
                    nc.tensor.matmul(
                        psc[:, OW : 2 * OW],
                        kvg[g][0:D, (k0 + 1) * 128 : (k0 + 2) * 128],
                        qg[p][:],
                        start=True,
                        stop=True,
                    )
                    pt = probs_pool.tile(
                        [128, 2 * OW], bf16, tag="pt", name=f"pt{p}_{kbp}"
                    )
                    nc.scalar.activation(pt[:], psc[:], EXP)
                    r = kbp - 2 * p
                    if r >= 0:
                        nc.vector.tensor_mul(
                            pt[:], pt[:], mask[:, r * 2 * OW : (r + 1) * 2 * OW]
                        )
                    pts.append(pt)
                return pts

            def cell_pv(p, g, pts):
                """PV accumulation of pair p for key-group g; ships output
                when the pair's causal range is complete."""
                if g == 0:
                    pouts[p] = ps_o.tile(
                        [D + 1, OW], f32, tag="pout", name=f"pout{p}"
                    )
                pout = pouts[p]
                for i, kbp in enumerate((2 * g, 2 * g + 1)):
                    k0 = (kbp % 2) * 2
                    pt = pts[i]
                    nc.tensor.matmul(
                        pout[:],
                        vsbg[g][:, k0, 0 : D + 1],
                        pt[:, 0:OW],
                        start=(kbp == 0),
                        stop=False,
                    )
                    nc.tensor.matmul(
                        pout[:],
                        vsbg[g][:, k0 + 1, 0 : D + 1],
                        pt[:, OW : 2 * OW],
                        start=False,
                        stop=(kbp == 2 * p + 1),
                    )
                if g == p:
                    # pair complete: ship out^T (row D = denominator)
                    otT = smalls.tile([D + 1, OW], f32, tag="otT", name=f"otT{p}")
                    nc.vector.tensor_copy(otT[:], pout[:])
                    nc.gpsimd.dma_start(out_d[p], otT[:])

            def cell(p, g):
                cell_pv(p, g, cell_scores(p, g))

            # x loads: group 0 first in 4 pieces (gates first matmul), then
            # group 3 (its pair has the most attention work), then 1, 2
            xgs[0] = xg_load(0, 4)
            xgs[3] = xg_load(3, 2)
            nc.sync.dma_start(cb[:, NE * 128 : CB_W], cb_d[:, NE * 128 : CB_W])
            nc.sync.dma_start(cf[:], cf_d[:])
            xgs[1] = xg_load(1, 2)
            xgs[2] = xg_load(2, 2)

            # interleaved schedule: spread pair-3 cells across the kernel so
            # the ScalarE exp stream never piles up into a serial tail; the
            # first cell's PV is deferred past proj(3) so the V-transpose
            # chain never blocks the in-order PE stream
            proj(0)
            proj(3)
            cell(0, 0)
            proj(1)
            cell(3, 0)
            cell(1, 0)
            cell(1, 1)
            proj(2)
            cell(3, 1)
            cell(2, 0)
            cell(2, 1)
            cell(2, 2)
            cell(3, 2)
            cell(3, 3)

    _close(nc)
    return nc


def _close(nc):
    nc.compile()


def _get_nc():
    if "full" not in _BUILT:
        _BUILT["full"] = _build()
    return _BUILT["full"]


def _host_inputs(x, Wq, bq, Wk, bk, Wv, bv):
    """Build the 8 per-core input maps."""
    import ml_dtypes

    bf = ml_dtypes.bfloat16
    x = np.asarray(x, np.float32)
    tri = np.triu(np.ones((128, 128), np.float32))  # [k,q]: 1 iff k <= q
    ones = np.ones((128, 128), np.float32)
    zeros = np.zeros((128, 128), np.float32)

    def wlayout(w, ncol):
        return np.ascontiguousarray(
            np.asarray(w, np.float32).reshape(NE, 128, ncol).transpose(1, 0, 2)
        ).reshape(128, NE * ncol)

    wkv_s = np.concatenate(
        [
            np.asarray(Wk, np.float32).reshape(NE, 128, D).transpose(1, 0, 2),
            np.asarray(Wv, np.float32).reshape(NE, 128, D).transpose(1, 0, 2),
        ],
        axis=2,
    ).reshape(128, NE * 128).astype(bf)
    wq_s = wlayout(np.asarray(Wq, np.float32) / float(D), D).astype(bf)
    bkv_s = np.concatenate(
        [np.asarray(bk, np.float32), np.asarray(bv, np.float32)]
    ).reshape(128, 1)
    bq_s = (np.asarray(bq, np.float32) / float(D)).reshape(D, 1)
    iden_s = np.zeros((128, D), np.float32)
    iden_s[np.arange(128), np.arange(128) % D] = 1.0
    iden_s = iden_s.astype(bf)

    # per-parity block permutation (within groups of 4), masks, xT layouts
    perm_idx = {}
    mask_h = {}
    for h in (0, 1):
        order = [4 * g + rel for g in range(SG) for rel in PERM_REL[h]]
        perm_idx[h] = np.concatenate(
            [np.arange(blk * 128, (blk + 1) * 128) for blk in order]
        )
        m = np.empty((128, 2, 2, OW), np.float32)
        for r in (0, 1):
            for j in (0, 1):
                krel = PERM_REL[h][2 * r + j]
                for qi in (0, 1):
                    qrel = PERM_REL[h][qi]
                    if krel < qrel:
                        sub = ones
                    elif krel == qrel:
                        sub = tri
                    else:
                        sub = zeros
                    m[:, r, j, qi * 128 : (qi + 1) * 128] = sub
        mask_h[h] = m.reshape(128, 2 * 2 * OW).astype(bf)

    in_maps = []
    xT_cache = {}
    for c in range(NCORES):
        b, h = c // 2, c % 2
        key = (b, h)
        if key not in xT_cache:
            xb = np.ascontiguousarray(
                x[b].T.reshape(NE, 128, S).transpose(1, 0, 2)
            )  # [128, NE, S]
            xp = xb[:, :, perm_idx[h]]  # permuted cols
            # layout [128, g, e, 512]
            xp = xp.reshape(128, NE, SG, GW).transpose(0, 2, 1, 3)
            xT_cache[key] = np.ascontiguousarray(xp).reshape(
                128, SG * NE * GW
            ).astype(bf)
        cf = np.zeros((128, 2), np.float32)
        cf[:, 0] = bkv_s[:, 0]
        cf[0:D, 1] = bq_s[:, 0]
        in_maps.append({
            "xT": xT_cache[key],
            "cb": np.concatenate([wkv_s, wq_s, mask_h[h], iden_s], axis=1),
            "cf": cf,
        })
    return in_maps


def _assemble(results):
    out = np.zeros((B, S, D), np.float32)
    for c in range(NCORES):
        b, h = c // 2, c % 2
        o = np.asarray(results[c]["out"], np.float32).reshape(SG, D + 1, OW)
        for g in range(SG):
            num, den = o[g, 0:D, :], o[g, D, :]
            for qi in (0, 1):
                blk = 4 * g + PERM_REL[h][qi]
                n = num[:, qi * 128 : (qi + 1) * 128]
                d_ = den[qi * 128 : (qi + 1) * 128]
                out[b, blk * 128 : (blk + 1) * 128] = (n / d_[None, :]).T
    return out


def kernel(x, Wq, bq, Wk, bk, Wv, bv):
    global LAST
    from concourse.bass_utils import run_bass_kernel_spmd

    nc = _get_nc()
    in_maps = _host_inputs(x, Wq, bq, Wk, bk, Wv, bv)
    LAST = run_bass_kernel_spmd(nc, in_maps, list(range(NCORES)))
    return _assemble(LAST.results)


# revision 37
# speedup vs baseline: 1.1511x; 1.1511x over previous
"""Causal single-head attention (B=4, S=2048, E=1024, D=64) on 8 TRN2 NeuronCores.

Sharding: core c -> batch b = c//2, parity h = c%2; core owns q-blocks of its
parity (h=1 even, h=0 odd; 68/136 causal units each). No collectives.

v2 design (vs v1 baseline at 86.3us):
- Per-core block-PERMUTED xT layout: within each group of 4 blocks the core's
  two owned q-blocks come first. The Q projection then only computes owned
  columns at FIXED offsets (uniform SPMD graph, no select ops), 1/2 the Q work.
  Causal group structure is preserved (permutation stays within groups of 4),
  masks/output mapping carry the permutation in DATA.
- [Wk|Wv] packed as one 128-wide stationary: K^T and V^T computed by a single
  pass over x (psum rows 0:64=K^T, 64:128=V^T), halving the K/V matmul count.
- Group-streamed pipeline: for each 512-token group g: DMA(g+1) || proj(g) ->
  attn(pair g). DMA hides behind compute; PE stays dense (HAM warm).
- V natural layout via dma_start_transpose (DMA engines), not PE transposes.
- exp on [128,512] tiles (2 key blocks per ACTIVATE) to amortize ScalarE
  instruction overhead -- attention was ScalarE-bound in v1.
- Epilogue: ships out^T = [65, 256] per pair (row 64 = softmax denominator);
  final transpose + normalize folded into the host-side unshard
  (flash-decoding style partial-result combine).
"""

import os
import sys

sys.path.insert(0, "/opt/trn_rl_repo")

import numpy as np

B, S, E, D = 4, 2048, 1024, 64
NB = S // 128       # 16 token blocks
NE = E // 128       # 8 contraction chunks
SG = 4              # groups of 4 blocks (512 tokens)
GW = 512            # group width (cols)
OW = 256            # owned q cols per group
NCORES = 8

# within-group block order: owned parity blocks first
PERM_REL = {1: [0, 2, 1, 3], 0: [1, 3, 0, 2]}

_BUILT = {}
LAST = None  # BassKernelResults of the most recent run (for test harness)


def _build():
    variant = os.environ.get("KVARIANT", "full")
    from concourse import bacc, bass, tile, mybir

    f32 = mybir.dt.float32
    bf16 = mybir.dt.bfloat16
    ADD = mybir.AluOpType.add
    EXP = mybir.ActivationFunctionType.Exp

    nc = bacc.Bacc(None, target_bir_lowering=False, debug=False)

    # cb: [wkv | wq | mask] packed; cf: [bkv | bq] packed
    CB_W = NE * 128 + NE * D + 2 * 2 * OW + D  # 1024 + 512 + 1024 + 64 (iden)
    xT_d = nc.declare_dram_parameter("xT", [128, SG * NE * GW], bf16, isOutput=False)
    cb_d = nc.declare_dram_parameter("cb", [128, CB_W], bf16, isOutput=False)
    cf_d = nc.declare_dram_parameter("cf", [128, 2], f32, isOutput=False)
    out_d = nc.declare_dram_parameter("out", [SG, D + 1, OW], f32, isOutput=True)
    if variant == "dump":
        dbg_d = nc.declare_dram_parameter("dbg", [SG, 128, GW + 4 * (D + 1)], bf16, isOutput=True)

    with tile.TileContext(nc) as tc:
        with (
            tc.tile_pool(name="consts", bufs=1) as consts,
            tc.tile_pool(name="xpool", bufs=1) as xpool,
            tc.tile_pool(name="acts", bufs=1) as acts,
            tc.tile_pool(name="probs", bufs=4) as probs_pool,
            tc.tile_pool(name="smalls", bufs=2) as smalls,
            tc.tile_pool(name="ps_a", bufs=3, space="PSUM") as ps_a,
            tc.tile_pool(name="ps_sc", bufs=3, space="PSUM") as ps_sc,
            tc.tile_pool(name="ps_o", bufs=2, space="PSUM") as ps_o,
        ):
            # ---- constants to SBUF: wkv first (gates the first matmul),
            # then group-0 x, then the rest of the constants
            cb = consts.tile([128, CB_W], bf16, tag="cb")
            cf = consts.tile([128, 2], f32, tag="cf")
            wkv = cb[:, 0 : NE * 128]
            wq = cb[:, NE * 128 : NE * 128 + NE * D]
            mask = cb[:, NE * 128 + NE * D : CB_W - D]
            iden = cb[:, CB_W - D : CB_W]
            bkv = cf[:, 0:1]
            bq = cf[0:D, 1:2]
            nc.sync.dma_start(wkv, cb_d[:, 0 : NE * 128])

            # warm the ScalarE exp table during initial DMA wait
            scr0 = smalls.tile([1, 1], f32, tag="scr0")
            scr1 = smalls.tile([1, 1], f32, tag="scr1")
            nc.vector.memset(scr0[:], 0.0)
            nc.scalar.activation(scr1[:], scr0[:], EXP)
            # warm the PE HAM clock gate with dummy matmuls while x loads
            wrm = smalls.tile([128, GW], bf16, tag="wrm")
            nc.vector.memset(wrm[:], 0.0)
            pwm = ps_a.tile([128, GW], f32, tag="ps", name="pwm")
            for i in range(7):
                nc.tensor.matmul(
                    pwm[:], wrm[:, 0:128], wrm[:], start=True, stop=True
                )

            # ---- persistent activations (per-group tiles -> exact deps)
            kvg = []   # [128, GW] bf16: rows 0:64 K^T, 64:128 V^T (permuted order)
            qg = []    # [64, OW] bf16: owned Q^T
            vsbg = []  # [128, 4, D+1] bf16: V natural per block + ones col
            for g in range(SG):
                kvg.append(acts.tile([128, GW], bf16, tag=f"kv{g}", name=f"kv{g}"))
                qg.append(acts.tile([D, OW], bf16, tag=f"q{g}", name=f"q{g}"))
                vsbg.append(acts.tile([128, 4, 128], bf16, tag=f"vsb{g}", name=f"vsb{g}"))

            for g in range(SG):
                nc.vector.memset(vsbg[g][:, :, D : D + 1], 1.0)

            def xg_load(g, nsplit):
                """Emit the load of group g's x^T slice (nsplit DMAs)."""
                xg = xpool.tile([128, NE, GW], bf16, tag=f"xg{g}", name=f"xg{g}")
                step = NE // nsplit
                for s in range(nsplit):
                    nc.sync.dma_start(
                        xg[:, s * step : (s + 1) * step, :],
                        xT_d[
                            :,
                            (g * NE + s * step) * GW : (g * NE + (s + 1) * step) * GW,
                        ],
                    )
                return xg

            xgs = {}
            pouts = {}

            def proj(g):
                """K|V packed pass + owned-Q pass for group g, plus the V
                natural-layout XBAR transpose chain (GpSimd bounce -> SP)."""
                xg = xgs[g]
                pkv = ps_a.tile([128, GW], f32, tag="ps", name=f"pkv{g}")
                for e in range(NE):
                    nc.tensor.matmul(
                        pkv[:],
                        wkv[:, e * 128 : (e + 1) * 128],
                        xg[:, e, :],
                        start=(e == 0),
                        stop=(e == NE - 1),
                    )
                nc.vector.tensor_scalar(kvg[g][:], pkv[:], bkv[:], None, ADD)
                pq = ps_a.tile([128, GW], f32, tag="ps", name=f"pq{g}")
                for e in range(NE):
                    nc.tensor.matmul(
                        pq[0:D, 0:OW],
                        wq[:, e * D : (e + 1) * D],
                        xg[:, e, 0:OW],
                        start=(e == 0),
                        stop=(e == NE - 1),
                    )
                nc.vector.tensor_scalar(qg[g][:], pq[0:D, 0:OW], bq[:], None, ADD)
                # V natural layout via PE transposes (row-group 64-127)
                for j in range(4):
                    ptr = ps_a.tile([128, GW], bf16, tag="ps", name=f"ptr{g}_{j}")
                    nc.tensor.transpose(
                        ptr[0:128, 0:D],
                        kvg[g][D:128, j * 128 : (j + 1) * 128],
                        iden[D:128, :],
                    )
                    nc.vector.tensor_copy(vsbg[g][:, j, 0:D], ptr[0:128, 0:D])
                if variant == "dump":
                    nc.sync.dma_start(dbg_d[g][:, 0:GW], kvg[g][:])
                    for j in range(4):
                        nc.sync.dma_start(
                            dbg_d[g][:, GW + j * (D + 1) : GW + (j + 1) * (D + 1)],
                            vsbg[g][:, j, 0 : D + 1],
                        )

            def cell_scores(p, g):
                """Scores+exp(+mask) of pair p vs key-group g -> pt tiles."""
                pts = []
                for kbp in (2 * g, 2 * g + 1):
                    k0 = (kbp % 2) * 2
                    psc = ps_sc.tile(
                        [128, 2 * OW], f32, tag="psc", name=f"psc{p}_{kbp}"
                    )
                    nc.tensor.matmul(
                        psc[:, 0:OW],
                        kvg[g][0:D, k0 * 128 : (k0 + 1) * 128],
                        qg[p][:],
                        start=True,
                        stop=True,
                    )
                    nc.tensor.matmul(
                        psc[:, OW : 2 * OW],
                        kvg[g][0:D, (k0 + 1) * 128 : (k0 + 2) * 128],
                        qg[p][:],
                        start=True,
                        stop=True,
                    )
                    pt = probs_pool.tile(
                        [128, 2 * OW], bf16, tag="pt", name=f"pt{p}_{kbp}"
                    )
                    nc.scalar.activation(pt[:], psc[:], EXP)
                    r = kbp - 2 * p
                    if r >= 0:
                        nc.vector.tensor_mul(
                            pt[:], pt[:], mask[:, r * 2 * OW : (r + 1) * 2 * OW]
                        )
                    pts.append(pt)
                return pts

            def cell_pv(p, g, pts):
                """PV accumulation of pair p for key-group g; ships output
                when the pair's causal range is complete."""
                if g == 0:
                    pouts[p] = ps_o.tile(
                        [D + 1, OW], f32, tag="pout", name=f"pout{p}"
                    )
                pout = pouts[p]
                for i, kbp in enumerate((2 * g, 2 * g + 1)):
                    k0 = (kbp % 2) * 2
                    pt = pts[i]
                    nc.tensor.matmul(
                        pout[:],
                        vsbg[g][:, k0, 0 : D + 1],
                        pt[:, 0:OW],
                        start=(kbp == 0),
                        stop=False,
                    )
                    nc.tensor.matmul(
                        pout[:],
                        vsbg[g][:, k0 + 1, 0 : D + 1],
                        pt[:, OW : 2 * OW],
                        start=False,
                        stop=(kbp == 2 * p + 1),
                    )
                if g == p:
                    # pair complete: ship out^T (row D = denominator)
                    otT = smalls.tile([D + 1, OW], f32, tag="otT", name=f"otT{p}")
                    nc.vector.tensor_copy(otT[:], pout[:])
                    nc.gpsimd.dma_start(out_d[p], otT[:])

            def cell(p, g):
                cell_pv(p, g, cell_scores(p, g))

            # x loads: group 0 first in 4 pieces (gates first matmul), then
            # group 3 (its pair has the most attention work), then 1, 2
            xgs[0] = xg_load(0, 4)
            xgs[3] = xg_load(3, 2)
            nc.sync.dma_start(cb[:, NE * 128 : CB_W], cb_d[:, NE * 128 : CB_W])
            nc.sync.dma_start(cf[:], cf_d[:])
            xgs[1] = xg_load(1, 2)
            xgs[2] = xg_load(2, 2)

            # interleaved schedule: spread pair-3 cells across the kernel so
            # the ScalarE exp stream never piles up into a serial tail; the
            # first cell's PV is deferred past proj(3) so the V-transpose
            # chain never blocks the in-order PE stream
            proj(0)
            proj(3)
            cell(0, 0)
            proj(1)
            cell(3, 0)
            cell(1, 0)
            cell(1, 1)
            proj(2)
            cell(3, 1)
            cell(2, 0)
            cell(2, 1)
            cell(2, 2)
            cell(3, 2)
            cell(3, 3)

    _close(nc)
    return nc


def _close(nc):
    nc.compile()


def _get_nc():
    if "full" not in _BUILT:
        _BUILT["full"] = _build()
    return _BUILT["full"]


def _host_inputs(x, Wq, bq, Wk, bk, Wv, bv):
    """Build the 8 per-core input maps."""
    import ml_dtypes

    bf = ml_dtypes.bfloat16
    x = np.asarray(x, np.float32)
    tri = np.triu(np.ones((128, 128), np.float32))  # [k,q]: 1 iff k <= q
    ones = np.ones((128, 128), np.float32)
    zeros = np.zeros((128, 128), np.float32)

    def wlayout(w, ncol):
        return np.ascontiguousarray(
            np.asarray(w, np.float32).reshape(NE, 128, ncol).transpose(1, 0, 2)
        ).reshape(128, NE * ncol)

    wkv_s = np.concatenate(
        [
            np.asarray(Wk, np.float32).reshape(NE, 128, D).transpose(1, 0, 2),
            np.asarray(Wv, np.float32).reshape(NE, 128, D).transpose(1, 0, 2),
        ],
        axis=2,
    ).reshape(128, NE * 128).astype(bf)
    wq_s = wlayout(np.asarray(Wq, np.float32) / float(D), D).astype(bf)
    bkv_s = np.concatenate(
        [np.asarray(bk, np.float32), np.asarray(bv, np.float32)]
    ).reshape(128, 1)
    bq_s = (np.asarray(bq, np.float32) / float(D)).reshape(D, 1)
    iden_s = np.zeros((128, D), np.float32)
    iden_s[np.arange(128), np.arange(128) % D] = 1.0
    iden_s = iden_s.astype(bf)

    # per-parity block permutation (within groups of 4), masks, xT layouts
    perm_idx = {}
    mask_h = {}
    for h in (0, 1):
        order = [4 * g + rel for g in range(SG) for rel in PERM_REL[h]]
        perm_idx[h] = np.concatenate(
            [np.arange(blk * 128, (blk + 1) * 128) for blk in order]
        )
        m = np.empty((128, 2, 2, OW), np.float32)
        for r in (0, 1):
            for j in (0, 1):
                krel = PERM_REL[h][2 * r + j]
                for qi in (0, 1):
                    qrel = PERM_REL[h][qi]
                    if krel < qrel:
                        sub = ones
                    elif krel == qrel:
                        sub = tri
                    else:
                        sub = zeros
                    m[:, r, j, qi * 128 : (qi + 1) * 128] = sub
        mask_h[h] = m.reshape(128, 2 * 2 * OW).astype(bf)

    in_maps = []
    xT_cache = {}
    for c in range(NCORES):
        b, h = c // 2, c % 2
        key = (b, h)
        if key not in xT_cache:
            xb = np.ascontiguousarray(
                x[b].T.reshape(NE, 128, S).transpose(1, 0, 2)
            )  # [128, NE, S]
            xp = xb[:, :, perm_idx[h]]  # permuted cols
            # layout [128, g, e, 512]
            xp = xp.reshape(128, NE, SG, GW).transpose(0, 2, 1, 3)
            xT_cache[key] = np.ascontiguousarray(xp).reshape(
                128, SG * NE * GW
            ).astype(bf)
        cf = np.zeros((128, 2), np.float32)
        cf[:, 0] = bkv_s[:, 0]
        cf[0:D, 1] = bq_s[:, 0]
        in_maps.append({
            "xT": xT_cache[key],
            "cb": np.concatenate([wkv_s, wq_s, mask_h[h], iden_s], axis=1),
            "cf": cf,
        })
    return in_maps


def _assemble(results):
    out = np.zeros((B, S, D), np.float32)
    for c in range(NCORES):
        b, h = c // 2, c % 2
        o = np.asarray(results[c]["out"], np.float32).reshape(SG, D + 1, OW)
        for g in range(SG):
            num, den = o[g, 0:D, :], o[g, D, :]
            for qi in (0, 1):
                blk = 4 * g + PERM_REL[h][qi]
                n = num[:, qi * 128 : (qi + 1) * 128]
                d_ = den[qi * 128 : (qi + 1) * 128]
                out[b, blk * 128 : (blk + 1) * 128] = (n / d_[None, :]).T
    return out


def kernel(x, Wq, bq, Wk, bk, Wv, bv):
    global LAST
    from concourse.bass_utils import run_bass_kernel_spmd

    nc = _get_nc()
    in_maps = _host_inputs(x, Wq, bq, Wk, bk, Wv, bv)
    LAST = run_bass_kernel_spmd(nc, in_maps, list(range(NCORES)))
    return _assemble(LAST.results)


# revision 38
# speedup vs baseline: 1.2399x; 1.0771x over previous
"""Causal single-head attention (B=4, S=2048, E=1024, D=64) on 8 TRN2 NeuronCores.

Sharding: core c -> batch b = c//2, parity h = c%2; core owns q-blocks of its
parity (h=1 even, h=0 odd; 68/136 causal units each). No collectives.

v2 design (vs v1 baseline at 86.3us):
- Per-core block-PERMUTED xT layout: within each group of 4 blocks the core's
  two owned q-blocks come first. The Q projection then only computes owned
  columns at FIXED offsets (uniform SPMD graph, no select ops), 1/2 the Q work.
  Causal group structure is preserved (permutation stays within groups of 4),
  masks/output mapping carry the permutation in DATA.
- [Wk|Wv] packed as one 128-wide stationary: K^T and V^T computed by a single
  pass over x (psum rows 0:64=K^T, 64:128=V^T), halving the K/V matmul count.
- Group-streamed pipeline: for each 512-token group g: DMA(g+1) || proj(g) ->
  attn(pair g). DMA hides behind compute; PE stays dense (HAM warm).
- V natural layout via dma_start_transpose (DMA engines), not PE transposes.
- exp on [128,512] tiles (2 key blocks per ACTIVATE) to amortize ScalarE
  instruction overhead -- attention was ScalarE-bound in v1.
- Epilogue: ships out^T = [65, 256] per pair (row 64 = softmax denominator);
  final transpose + normalize folded into the host-side unshard
  (flash-decoding style partial-result combine).
"""

import os
import sys

sys.path.insert(0, "/opt/trn_rl_repo")

import numpy as np

B, S, E, D = 4, 2048, 1024, 64
NB = S // 128       # 16 token blocks
NE = E // 128       # 8 contraction chunks
SG = 4              # groups of 4 blocks (512 tokens)
GW = 512            # group width (cols)
OW = 256            # owned q cols per group
NCORES = 8

# within-group block order: owned parity blocks first
PERM_REL = {1: [0, 2, 1, 3], 0: [1, 3, 0, 2]}

_BUILT = {}
LAST = None  # BassKernelResults of the most recent run (for test harness)


def _build():
    variant = os.environ.get("KVARIANT", "full")
    from concourse import bacc, bass, tile, mybir

    f32 = mybir.dt.float32
    bf16 = mybir.dt.bfloat16
    ADD = mybir.AluOpType.add
    EXP = mybir.ActivationFunctionType.Exp

    nc = bacc.Bacc(None, target_bir_lowering=False, debug=False)

    # cb: [wkv | wq | mask] packed; cf: [bkv | bq] packed
    CB_W = NE * 128 + NE * D + 2 * 2 * OW + D  # 1024 + 512 + 1024 + 64 (iden)
    xT_d = nc.declare_dram_parameter("xT", [128, SG * NE * GW], bf16, isOutput=False)
    cb_d = nc.declare_dram_parameter("cb", [128, CB_W], bf16, isOutput=False)
    cf_d = nc.declare_dram_parameter("cf", [128, 2], f32, isOutput=False)
    out_d = nc.declare_dram_parameter("out", [SG, D + 1, OW], f32, isOutput=True)
    if variant == "dump":
        dbg_d = nc.declare_dram_parameter("dbg", [SG, 128, GW + 4 * (D + 1)], bf16, isOutput=True)

    with tile.TileContext(nc) as tc:
        with (
            tc.tile_pool(name="consts", bufs=1) as consts,
            tc.tile_pool(name="xpool", bufs=1) as xpool,
            tc.tile_pool(name="acts", bufs=1) as acts,
            tc.tile_pool(name="probs", bufs=4) as probs_pool,
            tc.tile_pool(name="smalls", bufs=2) as smalls,
            tc.tile_pool(name="ps_a", bufs=3, space="PSUM") as ps_a,
            tc.tile_pool(name="ps_sc", bufs=3, space="PSUM") as ps_sc,
            tc.tile_pool(name="ps_o", bufs=2, space="PSUM") as ps_o,
        ):
            # ---- constants to SBUF: wkv first (gates the first matmul),
            # then group-0 x, then the rest of the constants
            cb = consts.tile([128, CB_W], bf16, tag="cb")
            cf = consts.tile([128, 2], f32, tag="cf")
            wkv = cb[:, 0 : NE * 128]
            wq = cb[:, NE * 128 : NE * 128 + NE * D]
            mask = cb[:, NE * 128 + NE * D : CB_W - D]
            iden = cb[:, CB_W - D : CB_W]
            bkv = cf[:, 0:1]
            bq = cf[0:D, 1:2]
            nc.sync.dma_start(wkv, cb_d[:, 0 : NE * 128])

            # warm the ScalarE exp table during initial DMA wait
            scr0 = smalls.tile([1, 1], f32, tag="scr0")
            scr1 = smalls.tile([1, 1], f32, tag="scr1")
            nc.vector.memset(scr0[:], 0.0)
            nc.scalar.activation(scr1[:], scr0[:], EXP)
            # warm the PE HAM clock gate with dummy matmuls while x loads
            wrm = smalls.tile([128, GW], bf16, tag="wrm")
            nc.vector.memset(wrm[:], 0.0)
            pwm = ps_a.tile([128, GW], f32, tag="ps", name="pwm")
            for i in range(7):
                nc.tensor.matmul(
                    pwm[:], wrm[:, 0:128], wrm[:], start=True, stop=True
                )

            # ---- persistent activations (per-group tiles -> exact deps)
            kvg = []   # [128, GW] bf16: rows 0:64 K^T, 64:128 V^T (permuted order)
            qg = []    # [64, OW] bf16: owned Q^T
            vsbg = []  # [128, 4, D+1] bf16: V natural per block + ones col
            for g in range(SG):
                kvg.append(acts.tile([128, GW], bf16, tag=f"kv{g}", name=f"kv{g}"))
                qg.append(acts.tile([D, OW], bf16, tag=f"q{g}", name=f"q{g}"))
                vsbg.append(acts.tile([128, 4, 128], bf16, tag=f"vsb{g}", name=f"vsb{g}"))

            for g in range(SG):
                nc.vector.memset(vsbg[g][:, :, D : D + 1], 1.0)

            def xg_load(g, nsplit):
                """Emit the load of group g's x^T slice (nsplit DMAs)."""
                xg = xpool.tile([128, NE, GW], bf16, tag=f"xg{g}", name=f"xg{g}")
                step = NE // nsplit
                for s in range(nsplit):
                    nc.sync.dma_start(
                        xg[:, s * step : (s + 1) * step, :],
                        xT_d[
                            :,
                            (g * NE + s * step) * GW : (g * NE + (s + 1) * step) * GW,
                        ],
                    )
                return xg

            xgs = {}
            pouts = {}

            def proj(g):
                """K|V packed pass + owned-Q pass for group g, plus the V
                natural-layout XBAR transpose chain (GpSimd bounce -> SP)."""
                xg = xgs[g]
                pkv = ps_a.tile([128, GW], f32, tag="ps", name=f"pkv{g}")
                for e in range(NE):
                    nc.tensor.matmul(
                        pkv[:],
                        wkv[:, e * 128 : (e + 1) * 128],
                        xg[:, e, :],
                        start=(e == 0),
                        stop=(e == NE - 1),
                    )
                nc.vector.tensor_scalar(kvg[g][:], pkv[:], bkv[:], None, ADD)
                pq = ps_a.tile([128, GW], f32, tag="ps", name=f"pq{g}")
                for e in range(NE):
                    nc.tensor.matmul(
                        pq[0:D, 0:OW],
                        wq[:, e * D : (e + 1) * D],
                        xg[:, e, 0:OW],
                        start=(e == 0),
                        stop=(e == NE - 1),
                    )
                nc.vector.tensor_scalar(qg[g][:], pq[0:D, 0:OW], bq[:], None, ADD)
                # V natural layout via PE transposes (row-group 64-127)
                for j in range(4):
                    ptr = ps_a.tile([128, GW], bf16, tag="ps", name=f"ptr{g}_{j}")
                    nc.tensor.transpose(
                        ptr[0:128, 0:D],
                        kvg[g][D:128, j * 128 : (j + 1) * 128],
                        iden[D:128, :],
                    )
                    nc.vector.tensor_copy(vsbg[g][:, j, 0:D], ptr[0:128, 0:D])
                if variant == "dump":
                    nc.sync.dma_start(dbg_d[g][:, 0:GW], kvg[g][:])
                    for j in range(4):
                        nc.sync.dma_start(
                            dbg_d[g][:, GW + j * (D + 1) : GW + (j + 1) * (D + 1)],
                            vsbg[g][:, j, 0 : D + 1],
                        )

            def cell_scores(p, g):
                """Scores+exp(+mask) of pair p vs key-group g -> pt tiles."""
                pts = []
                for kbp in (2 * g, 2 * g + 1):
                    k0 = (kbp % 2) * 2
                    psc = ps_sc.tile(
                        [128, 2 * OW], f32, tag="psc", name=f"psc{p}_{kbp}"
                    )
                    nc.tensor.matmul(
                        psc[:, 0:OW],
                        kvg[g][0:D, k0 * 128 : (k0 + 1) * 128],
                        qg[p][:],
                        start=True,
                        stop=True,
                    )
                    nc.tensor.matmul(
                        psc[:, OW : 2 * OW],
                        kvg[g][0:D, (k0 + 1) * 128 : (k0 + 2) * 128],
                        qg[p][:],
                        start=True,
                        stop=True,
                    )
                    pt = probs_pool.tile(
                        [128, 2 * OW], bf16, tag="pt", name=f"pt{p}_{kbp}"
                    )
                    nc.scalar.activation(pt[:], psc[:], EXP)
                    r = kbp - 2 * p
                    if r >= 0:
                        nc.vector.tensor_mul(
                            pt[:], pt[:], mask[:, r * 2 * OW : (r + 1) * 2 * OW]
                        )
                    pts.append(pt)
                return pts

            def cell_pv(p, g, pts):
                """PV accumulation of pair p for key-group g; ships output
                when the pair's causal range is complete."""
                if g == 0:
                    pouts[p] = ps_o.tile(
                        [D + 1, OW], f32, tag="pout", name=f"pout{p}"
                    )
                pout = pouts[p]
                for i, kbp in enumerate((2 * g, 2 * g + 1)):
                    k0 = (kbp % 2) * 2
                    pt = pts[i]
                    nc.tensor.matmul(
                        pout[:],
                        vsbg[g][:, k0, 0 : D + 1],
                        pt[:, 0:OW],
                        start=(kbp == 0),
                        stop=False,
                    )
                    nc.tensor.matmul(
                        pout[:],
                        vsbg[g][:, k0 + 1, 0 : D + 1],
                        pt[:, OW : 2 * OW],
                        start=False,
                        stop=(kbp == 2 * p + 1),
                    )
                if g == p:
                    # pair complete: ship out^T (row D = denominator)
                    otT = smalls.tile([D + 1, OW], f32, tag="otT", name=f"otT{p}")
                    nc.vector.tensor_copy(otT[:], pout[:])
                    nc.sync.dma_start(out_d[p], otT[:])

            def cell(p, g):
                cell_pv(p, g, cell_scores(p, g))

            # x loads: group 0 first in 4 pieces (gates first matmul), then
            # group 3 (its pair has the most attention work), then 1, 2
            xgs[0] = xg_load(0, 4)
            nc.sync.dma_start(cb[:, NE * 128 : CB_W], cb_d[:, NE * 128 : CB_W])
            nc.sync.dma_start(cf[:], cf_d[:])
            xgs[3] = xg_load(3, 2)
            xgs[1] = xg_load(1, 2)
            xgs[2] = xg_load(2, 2)

            # interleaved schedule: spread pair-3 cells across the kernel so
            # the ScalarE exp stream never piles up into a serial tail; the
            # first cell's PV is deferred past proj(3) so the V-transpose
            # chain never blocks the in-order PE stream
            proj(0)
            cell(0, 0)
            proj(3)
            proj(1)
            cell(3, 0)
            cell(1, 0)
            cell(1, 1)
            proj(2)
            cell(3, 1)
            cell(2, 0)
            cell(2, 1)
            cell(2, 2)
            cell(3, 2)
            cell(3, 3)

    _close(nc)
    return nc


def _close(nc):
    nc.compile()


def _get_nc():
    if "full" not in _BUILT:
        _BUILT["full"] = _build()
    return _BUILT["full"]


def _host_inputs(x, Wq, bq, Wk, bk, Wv, bv):
    """Build the 8 per-core input maps."""
    import ml_dtypes

    bf = ml_dtypes.bfloat16
    x = np.asarray(x, np.float32)
    tri = np.triu(np.ones((128, 128), np.float32))  # [k,q]: 1 iff k <= q
    ones = np.ones((128, 128), np.float32)
    zeros = np.zeros((128, 128), np.float32)

    def wlayout(w, ncol):
        return np.ascontiguousarray(
            np.asarray(w, np.float32).reshape(NE, 128, ncol).transpose(1, 0, 2)
        ).reshape(128, NE * ncol)

    wkv_s = np.concatenate(
        [
            np.asarray(Wk, np.float32).reshape(NE, 128, D).transpose(1, 0, 2),
            np.asarray(Wv, np.float32).reshape(NE, 128, D).transpose(1, 0, 2),
        ],
        axis=2,
    ).reshape(128, NE * 128).astype(bf)
    wq_s = wlayout(np.asarray(Wq, np.float32) / float(D), D).astype(bf)
    bkv_s = np.concatenate(
        [np.asarray(bk, np.float32), np.asarray(bv, np.float32)]
    ).reshape(128, 1)
    bq_s = (np.asarray(bq, np.float32) / float(D)).reshape(D, 1)
    iden_s = np.zeros((128, D), np.float32)
    iden_s[np.arange(128), np.arange(128) % D] = 1.0
    iden_s = iden_s.astype(bf)

    # per-parity block permutation (within groups of 4), masks, xT layouts
    perm_idx = {}
    mask_h = {}
    for h in (0, 1):
        order = [4 * g + rel for g in range(SG) for rel in PERM_REL[h]]
        perm_idx[h] = np.concatenate(
            [np.arange(blk * 128, (blk + 1) * 128) for blk in order]
        )
        m = np.empty((128, 2, 2, OW), np.float32)
        for r in (0, 1):
            for j in (0, 1):
                krel = PERM_REL[h][2 * r + j]
                for qi in (0, 1):
                    qrel = PERM_REL[h][qi]
                    if krel < qrel:
                        sub = ones
                    elif krel == qrel:
                        sub = tri
                    else:
                        sub = zeros
                    m[:, r, j, qi * 128 : (qi + 1) * 128] = sub
        mask_h[h] = m.reshape(128, 2 * 2 * OW).astype(bf)

    in_maps = []
    xT_cache = {}
    for c in range(NCORES):
        b, h = c // 2, c % 2
        key = (b, h)
        if key not in xT_cache:
            xb = np.ascontiguousarray(
                x[b].T.reshape(NE, 128, S).transpose(1, 0, 2)
            )  # [128, NE, S]
            xp = xb[:, :, perm_idx[h]]  # permuted cols
            # layout [128, g, e, 512]
            xp = xp.reshape(128, NE, SG, GW).transpose(0, 2, 1, 3)
            xT_cache[key] = np.ascontiguousarray(xp).reshape(
                128, SG * NE * GW
            ).astype(bf)
        cf = np.zeros((128, 2), np.float32)
        cf[:, 0] = bkv_s[:, 0]
        cf[0:D, 1] = bq_s[:, 0]
        in_maps.append({
            "xT": xT_cache[key],
            "cb": np.concatenate([wkv_s, wq_s, mask_h[h], iden_s], axis=1),
            "cf": cf,
        })
    return in_maps


def _assemble(results):
    out = np.zeros((B, S, D), np.float32)
    for c in range(NCORES):
        b, h = c // 2, c % 2
        o = np.asarray(results[c]["out"], np.float32).reshape(SG, D + 1, OW)
        for g in range(SG):
            num, den = o[g, 0:D, :], o[g, D, :]
            for qi in (0, 1):
                blk = 4 * g + PERM_REL[h][qi]
                n = num[:, qi * 128 : (qi + 1) * 128]
                d_ = den[qi * 128 : (qi + 1) * 128]
                out[b, blk * 128 : (blk + 1) * 128] = (n / d_[None, :]).T
    return out


def kernel(x, Wq, bq, Wk, bk, Wv, bv):
    global LAST
    from concourse.bass_utils import run_bass_kernel_spmd

    nc = _get_nc()
    in_maps = _host_inputs(x, Wq, bq, Wk, bk, Wv, bv)
    LAST = run_bass_kernel_spmd(nc, in_maps, list(range(NCORES)))
    return _assemble(LAST.results)
